# revision 6
# baseline (speedup 1.0000x reference)
"""Trainium2 Bass kernel for CustomBCEWithLogitsLoss (topk masking).

Math: with e = softplus(l) - l*t (elementwise BCE-with-logits),
  out = mean_all(e) + BCE_L * mean_{top20-by-logit per row}(e)
since top-k of sigmoid(logits) = top-k of logits, and the reference's
top-k BCE term equals e at those positions (-100 clamps never bind for
|l| < 100).

Per core (8-way batch shard, 512 rows = 4 tiles of [128, 10000]):
  DMA(SP HWDGE): L per tile (1x 5.12MB), T in 4 column chunks (1.28MB)
  ACT:    X_c = Exp(L_c); E_c = Ln(X_c+1) = softplus (accum -> sum sp);
          Copy(LT_c) (accum -> sum l*t)
  GPSIMD: LT_c = L_c * T_c; part of E_c -= LT_c
  DVE:    rest of E_c -= LT_c; 16x max8 over 625-col chunks -> 128
          candidates/row; 3x(max8 + match_replace) cascade -> tau = 20th
          largest logit; ME_c = (L_c >= tau) * E_c with accum -> masked sum
Exactness: per-chunk 8th-largest candidates (ch8) + 21st candidate (tau2)
are output; host flags rows where max(ch8) >= tau (candidate set may have
missed a top-20 value) or tau2 == tau (boundary tie) and recomputes those
rows exactly (expected ~1 row per ~1e5). Host combines partials in f64.
"""

import numpy as np

B, N, K = 4096, 10000, 20
NCORES = 8
R = B // NCORES          # rows per core
P = 128                  # partitions
NT = R // P              # tiles per core
CS = 2500                # streaming chunk width (4 chunks per row)
NCH = N // CS
CCH = 16                 # candidate chunks per row
W = N // CCH             # candidate chunk width (625)
SUBA = 1536              # columns of each E-subtract chunk done on DVE
SLOTS = 32               # per-tile output slots
NEG_INF = -1.0e30

_PROGRAM = None


def _build_program():
    import concourse.bacc as bacc
    import concourse.tile as tile
    import concourse.mybir as mybir

    nc = bacc.Bacc("TRN2", target_bir_lowering=False, debug=False)
    f32 = mybir.dt.float32
    logits = nc.dram_tensor("logits", [R, N], f32, kind="ExternalInput")
    targets = nc.dram_tensor("targets", [R, N], f32, kind="ExternalInput")
    out = nc.dram_tensor("partials", [P, NT * SLOTS], f32,
                         kind="ExternalOutput")
    Lr = logits.ap().rearrange("(t p) n -> t p n", p=P)
    Tr = targets.ap().rearrange("(t p) n -> t p n", p=P)

    AF = mybir.ActivationFunctionType
    OP = mybir.AluOpType

    with tile.TileContext(nc) as tc:
        with (
            tc.tile_pool(name="pL", bufs=2) as pL,
            tc.tile_pool(name="pE", bufs=1) as pE,
            tc.tile_pool(name="pT", bufs=2) as pT,
            tc.tile_pool(name="pX", bufs=1) as pX,
            tc.tile_pool(name="pLT", bufs=2) as pLT,
            tc.tile_pool(name="pME", bufs=1) as pME,
            tc.tile_pool(name="small", bufs=2) as small,
            tc.tile_pool(name="outp", bufs=1) as outp,
        ):
            OUT = outp.tile([P, NT * SLOTS], f32)
            nc.gpsimd.memset(OUT, 0.0)
            for t in range(NT):
                s0 = t * SLOTS
                Lt = pL.tile([P, N], f32, tag="L")
                nc.sync.dma_start(Lt, Lr[t])
                Ea = pE.tile([P, N // 2], f32, tag="Ea")
                Eb = pE.tile([P, N // 2], f32, tag="Eb")
                for c in range(NCH):
                    cl = c * CS
                    Eh = Ea if c < NCH // 2 else Eb
                    off = cl - (0 if c < NCH // 2 else N // 2)
                    Esl = Eh[:, off:off + CS]
                    Tc = pT.tile([P, CS], f32, tag="T")
                    nc.sync.dma_start(Tc, Tr[t][:, cl:cl + CS])
                    Xc = pX.tile([P, CS], f32, tag="X")
                    nc.scalar.activation(Xc, Lt[:, cl:cl + CS], AF.Exp)
                    # E_c = softplus = Ln(X+1); accum -> row sum of softplus
                    nc.scalar.activation(Esl, Xc, AF.Ln, bias=1.0, scale=1.0,
                                         accum_out=OUT[:, s0 + c:s0 + c + 1])
                    LTc = pLT.tile([P, CS], f32, tag="LT")
                    nc.gpsimd.tensor_mul(LTc, Lt[:, cl:cl + CS], Tc)
                    # row sum of l*t via ACT copy-accumulate (Xc is dead)
                    nc.scalar.activation(Xc, LTc, AF.Copy,
                                         accum_out=OUT[:, s0 + 4 + c:s0 + 5 + c])
                    # E_c -= LT_c, split between DVE and GPSIMD
                    nc.vector.tensor_sub(Esl[:, :SUBA], Esl[:, :SUBA],
                                         LTc[:, :SUBA])
                    nc.gpsimd.tensor_sub(Esl[:, SUBA:], Esl[:, SUBA:],
                                         LTc[:, SUBA:])

                # top-20 threshold: per-chunk top-8, then cascade on cand
                cand = small.tile([P, CCH * 8], f32, tag="cand")
                for c in range(CCH):
                    nc.vector.max(out=cand[:, c * 8:(c + 1) * 8],
                                  in_=Lt[:, c * W:(c + 1) * W])
                # 8th-largest of each chunk -> exactness check channel
                cv = cand[:].rearrange("p (c k) -> p c k", k=8)
                nc.gpsimd.tensor_copy(out=OUT[:, s0 + 16:s0 + 32],
                                      in_=cv[:, :, 7:8])
                m1 = small.tile([P, 8], f32, tag="m1")
                m2 = small.tile([P, 8], f32, tag="m2")
                m3 = small.tile([P, 8], f32, tag="m3")
                nc.vector.max(out=m1, in_=cand)
                nc.vector.match_replace(out=cand, in_to_replace=m1,
                                        in_values=cand, imm_value=NEG_INF)
                nc.vector.max(out=m2, in_=cand)
                nc.vector.match_replace(out=cand, in_to_replace=m2,
                                        in_values=cand, imm_value=NEG_INF)
                nc.vector.max(out=m3, in_=cand)
                tau = m3[:, 3:4]    # 20th largest; m3[:, 4] = 21st
                nc.gpsimd.tensor_copy(out=OUT[:, s0 + 12:s0 + 14],
                                      in_=m3[:, 3:5])

                # masked sum: ME_c = (L_c >= tau) * E_c, accum per chunk
                MEc = pME.tile([P, CS], f32, tag="ME")
                for c in range(NCH):
                    cl = c * CS
                    Eh = Ea if c < NCH // 2 else Eb
                    off = cl - (0 if c < NCH // 2 else N // 2)
                    nc.vector.scalar_tensor_tensor(
                        out=MEc, in0=Lt[:, cl:cl + CS], scalar=tau,
                        in1=Eh[:, off:off + CS],
                        op0=OP.is_ge, op1=OP.mult,
                        accum_out=OUT[:, s0 + 8 + c:s0 + 9 + c])

            nc.sync.dma_start(out.ap(), OUT)
    nc.compile()
    return nc


def _get_program():
    global _PROGRAM
    if _PROGRAM is None:
        _PROGRAM = _build_program()
    return _PROGRAM


def _run_on_cores(logits, targets, trace=False, **kw):
    from concourse import bass_utils
    nc = _get_program()
    in_maps = [
        {"logits": np.ascontiguousarray(logits[c * R:(c + 1) * R]),
         "targets": np.ascontiguousarray(targets[c * R:(c + 1) * R])}
        for c in range(NCORES)
    ]
    return bass_utils.run_bass_kernel_spmd(
        nc, in_maps, core_ids=list(range(NCORES)), trace=trace, **kw)


def _host_fix_rows(logits, targets, rows):
    """Exact per-row recompute of the top-20 term, replicating the
    reference's tie-breaking (top_k on f32 sigmoid, stable by index)."""
    out = {}
    for r in rows:
        l = logits[r].astype(np.float32)
        t = targets[r].astype(np.float64)
        p = (1.0 / (1.0 + np.exp(-l.astype(np.float64)))).astype(np.float32)
        idx = np.argsort(-p, kind="stable")[:K]
        ld = l[idx].astype(np.float64)
        td = t[idx]
        sp = np.maximum(ld, 0) + np.log1p(np.exp(-np.abs(ld)))
        out[r] = float(np.sum(sp - ld * td))
    return out


def kernel(logits, targets, BCE_L):
    logits = np.asarray(logits, dtype=np.float32)
    targets = np.asarray(targets, dtype=np.float32)
    res = _run_on_cores(logits, targets)
    # partials[core]: [P, NT*SLOTS]; global row = core*R + t*P + p
    bce_sum = 0.0
    me = np.zeros((NCORES, NT, P), dtype=np.float64)
    flag = np.zeros((NCORES, NT, P), dtype=bool)
    for c in range(NCORES):
        par = res.results[c]["partials"].astype(np.float64)
        for t in range(NT):
            s0 = t * SLOTS
            bce_sum += float(np.sum(par[:, s0:s0 + 4])
                             - np.sum(par[:, s0 + 4:s0 + 8]))
            me[c, t] = par[:, s0 + 8:s0 + 12].sum(axis=1)
            tau = par[:, s0 + 12]
            tau2 = par[:, s0 + 13]
            ch8max = par[:, s0 + 16:s0 + 32].max(axis=1)
            flag[c, t] = (ch8max >= tau) | (tau2 == tau)
    me_rows = me.reshape(-1)
    bad = np.nonzero(flag.reshape(-1))[0]
    if bad.size:
        fixes = _host_fix_rows(logits, targets, bad.tolist())
        for r, v in fixes.items():
            me_rows[r] = v
    out = bce_sum / (B * N) + float(BCE_L[0]) * float(me_rows.sum()) / (B * K)
    return np.array(out, dtype=np.float32)
